# revision 42
# baseline (speedup 1.0000x reference)
"""Trainium2 Bass kernel for a pre-LN transformer encoder block.

Problem: x[4, 2048, 512], H=8 heads, d_ff=2048, f32.
Sharding: 8 cores = (batch b, seq-half h). Each core computes the block for
1024 query rows of batch b; K/V are computed for the full 2048-row sequence
of that batch (duplicated across the pair) so no collectives are needed.
The host permutes each core's sequence so its own 1024 queries come first
(attention is permutation-invariant over keys).

v3 dataflow:
  - x ships feature-major fp8 (DoubleRow layout), token-major fp8 (LN1
    stats), and own-half f32 (residual, deferred DMA). Wq/Wk/Wv fp8 x SCL
    (Wq folds 1/sqrt(dk)); Wo/W1/W2 bf16 for accuracy.
  - LN1/LN2 rstd via Ln+Exp (one act-table set shared with softmax exp).
  - QKV as fp8 DoubleRow matmuls; the LN bias term (colsum(W) x bias)
    is accumulated into PSUM as a rank-1 f32r matmul, so the fixup is a
    single tensor_mul by the broadcast 1/(std*SCL) row (V: per-partition
    scalar mul on ScalarE).
  - scores bf16 per head with A/B head interleave (hides LDWEIGHTS in
    the other head's quadrant); exp on ScalarE in 1536-wide tiles into
    fp8 pg tiles laid out for DoubleRow attn@V over key pairs; softmax
    denominator via ones-column in V + reciprocal_approx_fast +
    partition_broadcast (all base-partition-0: HW requires it).
  - Wo bf16 token-major + residual; LN2; PE-transpose of xn2 feeds
    FFN1 bf16 1024-wide; relu on ScalarE; FFN2 bf16 + residual; store.
"""

import sys
import numpy as np

sys.path.insert(0, "/opt/trn_rl_repo")

B, S, D = 4, 2048, 512
H, DK, DFF = 8, 64, 2048
SQ = S // 2
P = 128
EPS = 1e-6
NJ = D // P           # 4 feature subtiles
NST = S // P          # 16 sequence tiles
NSQT = SQ // P        # 8 own-query tiles
NMT = DFF // P        # 16 ffn subtiles
SCL = 128.0           # fp8 weight scale

_CACHE = {}
_TRACE = {"trace": False, "trace_cores": None}
_LAST = {"res": None}


def _np_reference(x, src_mask, Wq, bq, Wk, bk, Wv, bv, Wo, bo,
                  W1, b1, W2, b2, g1, be1, g2, be2):
    """Faithful numpy fallback (used only for off-nominal inputs)."""
    x = np.asarray(x, np.float32)

    def ln(t, g, be):
        m = t.mean(-1, keepdims=True)
        var = ((t - m) ** 2).sum(-1, keepdims=True) / (t.shape[-1] - 1)
        return g * (t - m) / (np.sqrt(var) + EPS) + be

    Bv, Sv, _ = x.shape
    xn = ln(x, g1, be1)
    q = (xn @ Wq + bq).reshape(Bv, Sv, H, DK).transpose(0, 2, 1, 3)
    k = (xn @ Wk + bk).reshape(Bv, Sv, H, DK).transpose(0, 2, 1, 3)
    v = (xn @ Wv + bv).reshape(Bv, Sv, H, DK).transpose(0, 2, 1, 3)
    s = np.einsum("bhqd,bhkd->bhqk", q, k) / np.float32(np.sqrt(DK))
    s = np.where(np.asarray(src_mask) == 0, np.float32(-1e9), s)
    s = s - s.max(-1, keepdims=True)
    p = np.exp(s)
    p = p / p.sum(-1, keepdims=True)
    o = np.einsum("bhqk,bhkd->bhqd", p, v)
    o = o.transpose(0, 2, 1, 3).reshape(Bv, Sv, D)
    x = x + o @ Wo + bo
    xn = ln(x, g2, be2)
    return (x + np.maximum(xn @ W1 + b1, 0.0) @ W2 + b2).astype(np.float32)


def _build(g1, be1, g2, be2):
    import math
    import concourse.bass as bass
    import concourse.tile as tile
    from concourse import bacc, mybir
    from concourse.masks import make_identity
    from contextlib import ExitStack

    F32 = mybir.dt.float32
    F32R = mybir.dt.float32r
    BF16 = mybir.dt.bfloat16
    F8 = mybir.dt.float8e4
    AF = mybir.ActivationFunctionType
    MUL = mybir.AluOpType.mult
    ADD = mybir.AluOpType.add
    DR = mybir.MatmulPerfMode.DoubleRow

    nc = bacc.Bacc("TRN2", target_bir_lowering=False, debug=False)

    x8d = nc.dram_tensor("x8", [P, NJ, S], F8, kind="ExternalInput").ap()
    xtok8 = nc.dram_tensor("xtok8", [P, NST, D], F8,
                           kind="ExternalInput").ap()
    xtokf = nc.dram_tensor("xtokf", [P, NSQT, D], F32,
                           kind="ExternalInput").ap()
    Wq8 = nc.dram_tensor("Wq8", [P, NJ, D], F8, kind="ExternalInput").ap()
    Wk8 = nc.dram_tensor("Wk8", [P, NJ, D], F8, kind="ExternalInput").ap()
    Wv8 = nc.dram_tensor("Wv8", [P, NJ, D], F8, kind="ExternalInput").ap()
    Wo8 = nc.dram_tensor("Wo8", [P, NJ, D], BF16, kind="ExternalInput").ap()
    W18 = nc.dram_tensor("W18", [P, NJ, DFF], BF16, kind="ExternalInput").ap()
    W28 = nc.dram_tensor("W28", [P, NMT, D], BF16, kind="ExternalInput").ap()
    wqs = nc.dram_tensor("wqs", [1, D], F32, kind="ExternalInput").ap()
    wks = nc.dram_tensor("wks", [1, D], F32, kind="ExternalInput").ap()
    wvs = nc.dram_tensor("wvs", [1, D], F32, kind="ExternalInput").ap()

    out = nc.dram_tensor("out", [SQ, D], F32, kind="ExternalOutput").ap()
    scr_sc = nc.dram_tensor("scr_sc", [NST, P], F32)
    scr_bi = nc.dram_tensor("scr_bi", [NST, P], F32)

    with tile.TileContext(nc) as tc, ExitStack() as OU:
        res = OU.enter_context(tc.tile_pool(name="res", bufs=1))

        # ---------------- critical-path DMAs ----------------
        # kT/qT/x8 live in freeable pools: released before the FFN phase
        # so ff8 can reuse their SBUF.
        kT, qT, _frees = [], [], []
        for j in range(NJ):
            t, f = tc.tile([P, S], BF16, name=f"kT{j}")
            kT.append(t)
            _frees.append(f)
        for j in range(NJ):
            t, f = tc.tile([P, SQ], BF16, name=f"qT{j}")
            qT.append(t)
            _frees.append(f)
        x8, x8_free = tc.tile([P, NJ, S], F8, name="x8t")
        _frees.append(x8_free)
        nc.sync.dma_start(x8, x8d)
        Wk_t = res.tile([P, NJ, D], F8, name="Wk_t")
        nc.gpsimd.dma_start(Wk_t, Wk8)
        Wq_t = res.tile([P, NJ, D], F8, name="Wq_t")
        nc.gpsimd.dma_start(Wq_t, Wq8)
        Wv_t = res.tile([P, NJ, D], F8, name="Wv_t")
        nc.gpsimd.dma_start(Wv_t, Wv8)
        wqs_sb = res.tile([P, NJ], F32, name="wqs_sb")
        nc.sync.dma_start(wqs_sb, bass.AP(
            tensor=wqs.tensor, offset=wqs.offset, ap=[[1, P], [P, NJ]]))
        wks_sb = res.tile([P, NJ], F32, name="wks_sb")
        nc.sync.dma_start(wks_sb, bass.AP(
            tensor=wks.tensor, offset=wks.offset, ap=[[1, P], [P, NJ]]))
        wvs_b = res.tile([P, D], F32, name="wvs_b")
        nc.sync.dma_start(wvs_b, bass.AP(
            tensor=wvs.tensor, offset=wvs.offset, ap=[[0, P], [1, D]]))

        ident = res.tile([P, P], F32, name="ident")
        make_identity(nc, ident)
        identb = res.tile([P, P], BF16, name="identb")
        make_identity(nc, identb)

        # ---------------- persistent activations ----------------
        # V with ones column: [p, st, h*(dk+1)]
        vo8 = res.tile([P, NST, H * 65], BF16, name="vo8")
        oT8 = [res.tile([P, NJ, 512], BF16, name=f"oT8_{qc}")
               for qc in range(2)]
        scale_b = res.tile([P, S], F32, name="scale_b")
        bias_b = res.tile([P, S], F32, name="bias_b")
        x2tok = [res.tile([P, D], F32, name=f"x2t{sq}") for sq in range(NSQT)]
        xn2T8 = res.tile([P, NJ, SQ], BF16, name="xn2T8")
        mv = res.tile([P, 2, NST], F32, name="mv")
        mv2 = res.tile([P, 2, NSQT], F32, name="mv2")
        sc_all = res.tile([P, NST], F32, name="sc_all")
        sc2 = res.tile([P, NSQT], F32, name="sc2")
        bi2 = res.tile([P, NSQT], F32, name="bi2")
        sc_r = res.tile([NST, P], F32, name="sc_r")
        bi_r = res.tile([NST, P], F32, name="bi_r")

        # ones columns of vo8 (col 64 of each head's 65-wide strip)
        nc.gpsimd.memset(bass.AP(
            tensor=vo8.tensor, offset=vo8.offset + 64,
            ap=[vo8.ap[0], [H * 65, NST], [65, H]]), 1.0)

        # bulk weights/residual: DMAs deferred (emitted later)
        Wo_t = res.tile([P, NJ, D], BF16, name="Wo_t")
        W1_t = res.tile([P, NJ, DFF], BF16, name="W1_t")
        W2_t = res.tile([P, NMT, D], BF16, name="W2_t")
        xtf = res.tile([P, NSQT, D], F32, name="xtf")

        with ExitStack() as PH:
            p1 = PH.enter_context(
                tc.tile_pool(name="p1", bufs=8, space="PSUM"))
            p1s = PH.enter_context(tc.tile_pool(name="p1s", bufs=3))
            xtb, xtb_free = tc.tile([P, NST, D], F8, name="xtb")
            for c in range(4):
                nc.scalar.dma_start(xtb[:, 4 * c:4 * c + 4, :],
                                    xtok8[:, 4 * c:4 * c + 4, :])

            # ---------- LN1 stats (token-major fp8) ----------
            st6 = res.tile([P, NST, 6], F32, name="st6")
            for st in range(NST):
                nc.vector.bn_stats(st6[:, st, :], xtb[:, st, :])
                nc.vector.bn_aggr(mv[:, :, st:st + 1], st6[:, st, :])
            xtb_free()

            def rstd_newton(nx, dst, var_ap, gain, tmp_pool, n):
                # dst = gain * var'^(-1/2), var' = var*D/(D-1) ~ 1.0.
                # Quadratic seed + one Newton step (err ~1e-4 on [0.6,1.6]).
                vp = tmp_pool.tile([nx, n], F32, name="vp", tag="nw0",
                                   bufs=1)
                y0 = tmp_pool.tile([nx, n], F32, name="y0", tag="nw1",
                                   bufs=1)
                t = tmp_pool.tile([nx, n], F32, name="t", tag="nw2", bufs=1)
                nc.vector.tensor_scalar_mul(vp, var_ap, float(D) / (D - 1))
                nc.vector.tensor_scalar(y0, vp, 0.375, -1.25,
                                        op0=MUL, op1=ADD)
                nc.vector.tensor_mul(y0, y0, vp)
                nc.vector.tensor_scalar_add(y0, y0, 1.875)
                for _ in range(2):
                    nc.vector.tensor_mul(t, y0, y0)
                    nc.vector.tensor_mul(t, t, vp)
                    nc.vector.tensor_scalar(t, t, -0.5, 1.5,
                                            op0=MUL, op1=ADD)
                    nc.vector.tensor_mul(y0, y0, t)
                nc.vector.tensor_scalar_mul(dst, y0, gain)

            # transpose mean/var to row space [16,128], then all LN1
            # math there (keeps the PE queue free for QKV right after).
            mrows = p1s.tile([NST, P], F32, name="mrows", tag="mr", bufs=1)
            vrows = p1s.tile([NST, P], F32, name="vrows", tag="vr", bufs=1)
            for srcc, dstt in ((mv[:, 0, :], mrows), (mv[:, 1, :], vrows)):
                tp = p1.tile([NST, P], F32, name="tp", tag="p1")
                nc.tensor.transpose(tp, srcc, ident)
                nc.vector.tensor_copy(dstt, tp)
            # sc_r = g1/(std*SCL) rows; bi_r = -mean*g1/std + be1 rows
            rstd_newton(NST, sc_r, vrows, g1 / SCL, p1s, P)
            nc.vector.tensor_mul(bi_r, mrows, sc_r)
            nc.vector.tensor_scalar(bi_r, bi_r, -SCL, float(be1),
                                    op0=MUL, op1=ADD)
            # sc/bi cols for the V fixup (transpose back)
            tpb = p1.tile([P, NST], F32, name="tpb", tag="p1")
            nc.tensor.transpose(tpb, sc_r, ident[0:NST, 0:NST])
            nc.vector.tensor_copy(sc_all, tpb)
            bi_all = res.tile([P, NST], F32, name="bi_all")
            tpc = p1.tile([P, NST], F32, name="tpc", tag="p1")
            nc.tensor.transpose(tpc, bi_r, ident[0:NST, 0:NST])
            nc.vector.tensor_copy(bi_all, tpc)

            def bi_col(st):
                return bi_all[:, st:st + 1]
            for c in range(4):
                for rw, scr, dst in ((sc_r, scr_sc, scale_b),
                                     (bi_r, scr_bi, bias_b)):
                    nc.sync.dma_start(scr.ap()[4 * c:4 * c + 4, :],
                                      rw[4 * c:4 * c + 4, :])
                    nc.sync.dma_start(
                        dst[:, c * 512:(c + 1) * 512],
                        bass.AP(tensor=scr.ap().tensor,
                                offset=scr.ap().offset + c * 512,
                                ap=[[0, P], [1, 512]]))

            # deferred bulk DMAs (behind the critical loads on each queue)
            nc.gpsimd.dma_start(Wo_t, Wo8)
            nc.gpsimd.dma_start(W1_t, W18)
            nc.gpsimd.dma_start(W2_t, W28)
            nc.sync.dma_start(xtf, xtokf)

            # ---------- QKV (pure fp8 DoubleRow; fixup on DVE/Scalar) ---
            def qk_feat(Wt, wsum, dstT, j, c):
                # feature-major: out[dim 128, tok 512]
                ps = p1.tile([P, 512], F32, name="ps_qk", tag="p1")
                for i in range(2):
                    nc.tensor.matmul(
                        ps, Wt[:, 2 * i:2 * i + 2, j * P:(j + 1) * P],
                        x8[:, 2 * i:2 * i + 2, c * 512:(c + 1) * 512],
                        start=(i == 0), stop=(i == 1), perf_mode=DR)
                t = p1s.tile([P, 512], F32, name="fx", tag="fx")
                nc.vector.tensor_mul(t, ps,
                                     scale_b[:, c * 512:(c + 1) * 512])
                nc.vector.scalar_tensor_tensor(
                    dstT[j][:, c * 512:(c + 1) * 512],
                    bias_b[:, c * 512:(c + 1) * 512],
                    wsum[:, j:j + 1], t, op0=MUL, op1=ADD)

            def v_tok(st):
                # token-major: out[tok 128, vdim 512]
                ps = p1.tile([P, D], F32, name="ps_v", tag="p1")
                for i in range(2):
                    nc.tensor.matmul(
                        ps, x8[:, 2 * i:2 * i + 2, st * P:(st + 1) * P],
                        Wv_t[:, 2 * i:2 * i + 2, :],
                        start=(i == 0), stop=(i == 1), perf_mode=DR)
                t = p1s.tile([P, D], F32, name="fxv", tag="fx")
                nc.scalar.activation(t, ps, AF.Copy,
                                     scale=sc_all[:, st:st + 1])
                dst = bass.AP(
                    tensor=vo8.tensor, offset=vo8.offset + st * H * 65,
                    ap=[vo8.ap[0], [65, H], [1, DK]])
                nc.vector.scalar_tensor_tensor(
                    dst, wvs_b.rearrange("p (h c) -> p h c", c=DK),
                    bi_col(st), t.rearrange("p (h c) -> p h c", c=DK),
                    op0=MUL, op1=ADD)

            for c in range(4):
                qk_feat(Wk_t, wks_sb, kT, 0, c)
            for c in range(2):
                qk_feat(Wq_t, wqs_sb, qT, 0, c)
            for st in range(6):
                v_tok(st)
            for j in range(1, NJ):
                for c in range(4):
                    qk_feat(Wk_t, wks_sb, kT, j, c)
                for c in range(2):
                    qk_feat(Wq_t, wqs_sb, qT, j, c)
                for st in range(6 + (j - 1) * 4, min(NST, 6 + j * 4)):
                    v_tok(st)
            for st in range(14, NST):
                v_tok(st)

        # ================= attention =================
        GRP = [(0, 3), (3, 3), (6, 3), (9, 3), (12, 3), (15, 1)]
        with ExitStack() as PA:
            ps_sg = PA.enter_context(
                tc.tile_pool(name="ps_sg", bufs=2, space="PSUM"))
            ps_acc = PA.enter_context(
                tc.tile_pool(name="ps_acc", bufs=2, space="PSUM"))
            sb_pg = PA.enter_context(tc.tile_pool(name="sb_pg", bufs=2))
            sb_nrm = PA.enter_context(tc.tile_pool(name="sb_nrm", bufs=1))

            for qc in range(2):
                for hp in range(4):
                    accs = [ps_acc.tile([65, 512], F32, name=f"acc{h2}",
                                        tag="acc") for h2 in range(2)]

                    def attnv(g0, gn, pgs):
                        for i in range(gn):
                            kt = g0 + i
                            for h2 in range(2):
                                h = 2 * hp + h2
                                nc.tensor.matmul(
                                    accs[h2],
                                    vo8[:, kt, h * 65:h * 65 + 65],
                                    pgs[h2][:, i, :],
                                    start=(kt == 0), stop=(kt == NST - 1))

                    pend = None
                    for g0, gn in GRP:
                        pgs = [sb_pg.tile([P, gn, 512], BF16,
                                          name=f"pg{h2}", tag=f"pg{h2}")
                               for h2 in range(2)]
                        for h2 in range(2):
                            bp = 64 * h2
                            sg = ps_sg.tile([P, gn, 512], F32, name="sg",
                                            tag="sg")
                            for i in range(gn):
                                kt = g0 + i
                                nc.tensor.matmul(
                                    sg[:, i, :],
                                    kT[hp][bp:bp + DK,
                                           kt * P:(kt + 1) * P],
                                    qT[hp][bp:bp + DK,
                                           qc * 512:(qc + 1) * 512])
                            nc.scalar.activation(
                                pgs[h2].rearrange("p a b -> p (a b)"),
                                sg.rearrange("p a b -> p (a b)"), AF.Exp)
                        if pend is not None:
                            attnv(*pend)
                        pend = (g0, gn, pgs)
                    attnv(*pend)
                    # normalize: all base-partition-0 (HW requirement for
                    # custom-DVE/partition_broadcast); only DVE out shifted.
                    for h2 in range(2):
                        den = sb_nrm.tile([1, 512], F32, name="den",
                                          tag=f"den{h2}")
                        rec = sb_nrm.tile([1, 512], F32, name="rec",
                                          tag=f"rec{h2}")
                        rbh = sb_nrm.tile([64, 512], F32, name="rbh",
                                          tag=f"rb{h2}")
                        nc.vector.tensor_copy(den[0:1, :],
                                              accs[h2][DK:DK + 1, :])
                        nc.vector.reciprocal_approx_fast(rec[0:1, :],
                                                         den[0:1, :])
                        nc.gpsimd.partition_broadcast(
                            rbh[0:64, :], rec[0:1, :], channels=64)
                        nc.vector.tensor_mul(
                            oT8[qc][64 * h2:64 * h2 + 64, hp, :],
                            accs[h2][0:DK, :], rbh[0:64, :])

        for f in reversed(_frees):
            f()

        # ================= Wo + LN2 + FFN =================
        with ExitStack() as PF:
            pf = PF.enter_context(
                tc.tile_pool(name="pf", bufs=4, space="PSUM"))
            pf1 = PF.enter_context(
                tc.tile_pool(name="pf1", bufs=2, space="PSUM"))
            fs = PF.enter_context(tc.tile_pool(name="fs", bufs=2))
            ff8 = fs.tile([P, NMT, SQ], BF16, name="ff8", tag="ff", bufs=1)

            # Wo token-major + residual + LN2 stats (both qc)
            for sq in range(NSQT):
                qc, sl = sq // 4, sq % 4
                ps = pf.tile([P, D], F32, name="ps_wo", tag="f")
                for j in range(NJ):
                    nc.tensor.matmul(
                        ps, oT8[qc][:, j, sl * P:(sl + 1) * P],
                        Wo_t[:, j, :], start=(j == 0), stop=(j == NJ - 1))
                nc.vector.tensor_add(x2tok[sq], ps, xtf[:, sq, :])
                st6b = fs.tile([P, 6], F32, name="st6b", tag="st6b")
                nc.vector.bn_stats(st6b, x2tok[sq])
                nc.vector.bn_aggr(mv2[:, :, sq:sq + 1], st6b)
            # LN2 scale/bias (per-token = per-partition), all 8 tiles
            rstd_newton(P, sc2, mv2[:, 1, :], g2, fs, NSQT)
            nc.vector.tensor_mul(bi2, mv2[:, 0, :], sc2)
            nc.vector.tensor_scalar(bi2, bi2, -1.0, float(be2),
                                    op0=MUL, op1=ADD)
            # LN2 apply + transpose, pipelined with FFN1 per 512-chunk
            for c in range(2):
                for sl in range(4):
                    sq = c * 4 + sl
                    xn2 = fs.tile([P, D], BF16, name="xn2", tag="xn2")
                    nc.vector.tensor_scalar(xn2, x2tok[sq],
                                            sc2[:, sq:sq + 1],
                                            bi2[:, sq:sq + 1],
                                            op0=MUL, op1=ADD)
                    for j in range(NJ):
                        pt = pf.tile([P, P], BF16, name="ptt", tag="f")
                        nc.tensor.transpose(pt, xn2[:, j * P:(j + 1) * P],
                                            identb)
                        nc.vector.tensor_copy(
                            xn2T8[:, j, sq * P:(sq + 1) * P], pt)
                for mt in range(NMT):
                    ps = pf1.tile([P, 512], F32, name="ps_f1", tag="f1")
                    for j in range(NJ):
                        nc.tensor.matmul(
                            ps, W1_t[:, j, mt * P:(mt + 1) * P],
                            xn2T8[:, j, c * 512:(c + 1) * 512],
                            start=(j == 0), stop=(j == NJ - 1))
                    nc.scalar.activation(
                        ff8[:, mt, c * 512:(c + 1) * 512], ps, AF.Relu)
            # FFN2 (bf16) + residual + store
            for sq in range(NSQT):
                ps = pf.tile([P, D], F32, name="ps_f2", tag="f")
                for mt in range(NMT):
                    nc.tensor.matmul(
                        ps, ff8[:, mt, sq * P:(sq + 1) * P],
                        W2_t[:, mt, :],
                        start=(mt == 0), stop=(mt == NMT - 1))
                ot = fs.tile([P, D], F32, name="ot", tag="ot")
                nc.vector.tensor_add(ot, ps, x2tok[sq])
                nc.sync.dma_start(out[sq * P:(sq + 1) * P, :], ot)

    nc.compile()
    return nc


def _fast_path_ok(inputs):
    if not np.all(np.asarray(inputs["src_mask"]) != 0):
        return False
    for b in ("bq", "bk", "bv", "bo", "b1", "b2"):
        if np.any(np.asarray(inputs[b]) != 0):
            return False
    if float(np.asarray(inputs["g1"])) <= 0 or float(np.asarray(inputs["g2"])) <= 0:
        return False
    return True


def _fp8(a):
    import ml_dtypes
    return np.ascontiguousarray(
        np.clip(a, -240.0, 240.0).astype(ml_dtypes.float8_e4m3))


def _w_layout(w, nsub):
    # [K, M] -> [128, nsub, M] with k = j*128 + p
    k, m = w.shape
    return np.ascontiguousarray(
        w.reshape(nsub, P, m).transpose(1, 0, 2))


def kernel(**inputs):
    x = np.ascontiguousarray(np.asarray(inputs["x"], np.float32))
    g1 = float(np.asarray(inputs["g1"]))
    be1 = float(np.asarray(inputs["be1"]))
    g2 = float(np.asarray(inputs["g2"]))
    be2 = float(np.asarray(inputs["be2"]))

    if not _fast_path_ok(inputs):
        return _np_reference(**{k: np.asarray(v) for k, v in inputs.items()})

    from concourse.bass_utils import run_bass_kernel_spmd
    import ml_dtypes

    key = (g1, be1, g2, be2)
    if key not in _CACHE:
        _CACHE[key] = _build(*key)
    nc = _CACHE[key]

    scale = np.float32(1.0 / np.sqrt(DK))
    Wq8 = _fp8(_w_layout(np.asarray(inputs["Wq"], np.float32) * (scale * SCL), NJ))
    Wk8 = _fp8(_w_layout(np.asarray(inputs["Wk"], np.float32) * SCL, NJ))
    Wv8 = _fp8(_w_layout(np.asarray(inputs["Wv"], np.float32) * SCL, NJ))
    BFD = ml_dtypes.bfloat16
    Wo8 = np.ascontiguousarray(
        _w_layout(np.asarray(inputs["Wo"], np.float32), NJ).astype(BFD))
    W18 = np.ascontiguousarray(
        _w_layout(np.asarray(inputs["W1"], np.float32), NJ).astype(BFD))
    W28 = np.ascontiguousarray(
        _w_layout(np.asarray(inputs["W2"], np.float32), NMT).astype(BFD))

    def colsum(w8):
        return np.ascontiguousarray(
            (w8.astype(np.float32).sum(axis=(0, 1)) / SCL)[None, :])

    wqs = colsum(Wq8)
    wks = colsum(Wk8)
    wvs = colsum(Wv8)

    in_maps = []
    for c in range(8):
        b, hh = c // 2, c % 2
        if hh == 0:
            xp = x[b]
        else:
            xp = np.concatenate([x[b, SQ:], x[b, :SQ]], axis=0)
        xp = np.ascontiguousarray(xp)
        x8 = _fp8(_w_layout(xp.T.copy(), NJ))
        xt8 = _fp8(xp.reshape(NST, P, D).transpose(1, 0, 2))
        xtf_l = np.ascontiguousarray(
            xp[:SQ].reshape(NSQT, P, D).transpose(1, 0, 2))
        in_maps.append(dict(
            x8=np.ascontiguousarray(x8),
            xtok8=xt8,
            xtokf=xtf_l,
            Wq8=Wq8, Wk8=Wk8, Wv8=Wv8, Wo8=Wo8, W18=W18, W28=W28,
            wqs=wqs, wks=wks, wvs=wvs))

    res = run_bass_kernel_spmd(nc, in_maps, core_ids=list(range(8)),
                               trace=_TRACE["trace"],
                               trace_cores=_TRACE["trace_cores"])
    _LAST["res"] = res

    full = np.empty((B, S, D), np.float32)
    for c in range(8):
        b, hh = c // 2, c % 2
        full[b, hh * SQ:(hh + 1) * SQ] = res.results[c]["out"]
    return full


# revision 43
# speedup vs baseline: 1.0895x; 1.0895x over previous
"""Trainium2 Bass kernel for a pre-LN transformer encoder block.

Problem: x[4, 2048, 512], H=8 heads, d_ff=2048, f32.
Sharding: 8 cores = (batch b, seq-half h). Each core computes the block for
1024 query rows of batch b; K/V are computed for the full 2048-row sequence
of that batch (duplicated across the pair) so no collectives are needed.
The host permutes each core's sequence so its own 1024 queries come first
(attention is permutation-invariant over keys).

v3 dataflow:
  - x ships feature-major fp8 (DoubleRow layout), token-major fp8 (LN1
    stats), and own-half f32 (residual, deferred DMA). Wq/Wk/Wv fp8 x SCL
    (Wq folds 1/sqrt(dk)); Wo/W1/W2 bf16 for accuracy.
  - LN1/LN2 rstd via Ln+Exp (one act-table set shared with softmax exp).
  - QKV as fp8 DoubleRow matmuls; the LN bias term (colsum(W) x bias)
    is accumulated into PSUM as a rank-1 f32r matmul, so the fixup is a
    single tensor_mul by the broadcast 1/(std*SCL) row (V: per-partition
    scalar mul on ScalarE).
  - scores bf16 per head with A/B head interleave (hides LDWEIGHTS in
    the other head's quadrant); exp on ScalarE in 1536-wide tiles into
    fp8 pg tiles laid out for DoubleRow attn@V over key pairs; softmax
    denominator via ones-column in V + reciprocal_approx_fast +
    partition_broadcast (all base-partition-0: HW requires it).
  - Wo bf16 token-major + residual; LN2; PE-transpose of xn2 feeds
    FFN1 bf16 1024-wide; relu on ScalarE; FFN2 bf16 + residual; store.
"""

import sys
import numpy as np

sys.path.insert(0, "/opt/trn_rl_repo")

B, S, D = 4, 2048, 512
H, DK, DFF = 8, 64, 2048
SQ = S // 2
P = 128
EPS = 1e-6
NJ = D // P           # 4 feature subtiles
NST = S // P          # 16 sequence tiles
NSQT = SQ // P        # 8 own-query tiles
NMT = DFF // P        # 16 ffn subtiles
SCL = 128.0           # fp8 weight scale

_CACHE = {}
_TRACE = {"trace": False, "trace_cores": None}
_LAST = {"res": None}


def _np_reference(x, src_mask, Wq, bq, Wk, bk, Wv, bv, Wo, bo,
                  W1, b1, W2, b2, g1, be1, g2, be2):
    """Faithful numpy fallback (used only for off-nominal inputs)."""
    x = np.asarray(x, np.float32)

    def ln(t, g, be):
        m = t.mean(-1, keepdims=True)
        var = ((t - m) ** 2).sum(-1, keepdims=True) / (t.shape[-1] - 1)
        return g * (t - m) / (np.sqrt(var) + EPS) + be

    Bv, Sv, _ = x.shape
    xn = ln(x, g1, be1)
    q = (xn @ Wq + bq).reshape(Bv, Sv, H, DK).transpose(0, 2, 1, 3)
    k = (xn @ Wk + bk).reshape(Bv, Sv, H, DK).transpose(0, 2, 1, 3)
    v = (xn @ Wv + bv).reshape(Bv, Sv, H, DK).transpose(0, 2, 1, 3)
    s = np.einsum("bhqd,bhkd->bhqk", q, k) / np.float32(np.sqrt(DK))
    s = np.where(np.asarray(src_mask) == 0, np.float32(-1e9), s)
    s = s - s.max(-1, keepdims=True)
    p = np.exp(s)
    p = p / p.sum(-1, keepdims=True)
    o = np.einsum("bhqk,bhkd->bhqd", p, v)
    o = o.transpose(0, 2, 1, 3).reshape(Bv, Sv, D)
    x = x + o @ Wo + bo
    xn = ln(x, g2, be2)
    return (x + np.maximum(xn @ W1 + b1, 0.0) @ W2 + b2).astype(np.float32)


def _build(g1, be1, g2, be2):
    import math
    import concourse.bass as bass
    import concourse.tile as tile
    from concourse import bacc, mybir
    from concourse.masks import make_identity
    from contextlib import ExitStack

    F32 = mybir.dt.float32
    F32R = mybir.dt.float32r
    BF16 = mybir.dt.bfloat16
    F8 = mybir.dt.float8e4
    AF = mybir.ActivationFunctionType
    MUL = mybir.AluOpType.mult
    ADD = mybir.AluOpType.add
    DR = mybir.MatmulPerfMode.DoubleRow

    nc = bacc.Bacc("TRN2", target_bir_lowering=False, debug=False)

    x8d = nc.dram_tensor("x8", [P, NJ, S], F8, kind="ExternalInput").ap()
    xtok8 = nc.dram_tensor("xtok8", [P, NST, D], F8,
                           kind="ExternalInput").ap()
    xtokf = nc.dram_tensor("xtokf", [P, NSQT, D], F32,
                           kind="ExternalInput").ap()
    Wq8 = nc.dram_tensor("Wq8", [P, NJ, D], F8, kind="ExternalInput").ap()
    Wk8 = nc.dram_tensor("Wk8", [P, NJ, D], F8, kind="ExternalInput").ap()
    Wv8 = nc.dram_tensor("Wv8", [P, NJ, D], F8, kind="ExternalInput").ap()
    Wo8 = nc.dram_tensor("Wo8", [P, NJ, D], BF16, kind="ExternalInput").ap()
    W18 = nc.dram_tensor("W18", [P, NJ, DFF], BF16, kind="ExternalInput").ap()
    W28 = nc.dram_tensor("W28", [P, NMT, D], BF16, kind="ExternalInput").ap()
    wqs = nc.dram_tensor("wqs", [1, D], F32, kind="ExternalInput").ap()
    wks = nc.dram_tensor("wks", [1, D], F32, kind="ExternalInput").ap()
    wvs = nc.dram_tensor("wvs", [1, D], F32, kind="ExternalInput").ap()

    out = nc.dram_tensor("out", [SQ, D], F32, kind="ExternalOutput").ap()
    scr_sc = nc.dram_tensor("scr_sc", [NST, P], F32)
    scr_bi = nc.dram_tensor("scr_bi", [NST, P], F32)

    with tile.TileContext(nc) as tc, ExitStack() as OU:
        res = OU.enter_context(tc.tile_pool(name="res", bufs=1))

        # ---------------- critical-path DMAs ----------------
        # kT/qT/x8 live in freeable pools: released before the FFN phase
        # so ff8 can reuse their SBUF.
        kT, qT, _frees = [], [], []
        for j in range(NJ):
            t, f = tc.tile([P, S], BF16, name=f"kT{j}")
            kT.append(t)
            _frees.append(f)
        for j in range(NJ):
            t, f = tc.tile([P, SQ], BF16, name=f"qT{j}")
            qT.append(t)
            _frees.append(f)
        x8, x8_free = tc.tile([P, NJ, S], F8, name="x8t")
        _frees.append(x8_free)
        nc.sync.dma_start(x8, x8d)
        Wk_t = res.tile([P, NJ, D], F8, name="Wk_t")
        nc.gpsimd.dma_start(Wk_t, Wk8)
        Wq_t = res.tile([P, NJ, D], F8, name="Wq_t")
        nc.gpsimd.dma_start(Wq_t, Wq8)
        Wv_t = res.tile([P, NJ, D], F8, name="Wv_t")
        nc.gpsimd.dma_start(Wv_t, Wv8)
        wqs_sb = res.tile([P, NJ], F32, name="wqs_sb")
        nc.sync.dma_start(wqs_sb, bass.AP(
            tensor=wqs.tensor, offset=wqs.offset, ap=[[1, P], [P, NJ]]))
        wks_sb = res.tile([P, NJ], F32, name="wks_sb")
        nc.sync.dma_start(wks_sb, bass.AP(
            tensor=wks.tensor, offset=wks.offset, ap=[[1, P], [P, NJ]]))
        wvs_b = res.tile([P, D], F32, name="wvs_b")
        nc.sync.dma_start(wvs_b, bass.AP(
            tensor=wvs.tensor, offset=wvs.offset, ap=[[0, P], [1, D]]))

        ident = res.tile([P, P], F32, name="ident")
        make_identity(nc, ident)
        identb = res.tile([P, P], BF16, name="identb")
        make_identity(nc, identb)

        # ---------------- persistent activations ----------------
        # V with ones column: [p, st, h*(dk+1)]
        vo8 = res.tile([P, NST, H * 65], BF16, name="vo8")
        oT8 = [res.tile([P, NJ, 512], BF16, name=f"oT8_{qc}")
               for qc in range(2)]
        scale_b = res.tile([P, S], F32, name="scale_b")
        bias_b = res.tile([P, S], F32, name="bias_b")
        x2tok = [res.tile([P, D], F32, name=f"x2t{sq}") for sq in range(NSQT)]
        xn2T8 = res.tile([P, NJ, SQ], BF16, name="xn2T8")
        mv = res.tile([P, 2, NST], F32, name="mv")
        mv2 = res.tile([P, 2, NSQT], F32, name="mv2")
        sc_all = res.tile([P, NST], F32, name="sc_all")
        sc2 = res.tile([P, NSQT], F32, name="sc2")
        bi2 = res.tile([P, NSQT], F32, name="bi2")
        sc_r = res.tile([NST, P], F32, name="sc_r")
        bi_r = res.tile([NST, P], F32, name="bi_r")

        # ones columns of vo8 (col 64 of each head's 65-wide strip)
        nc.gpsimd.memset(bass.AP(
            tensor=vo8.tensor, offset=vo8.offset + 64,
            ap=[vo8.ap[0], [H * 65, NST], [65, H]]), 1.0)

        # bulk weights/residual: DMAs deferred (emitted later)
        Wo_t = res.tile([P, NJ, D], BF16, name="Wo_t")
        W1_t = res.tile([P, NJ, DFF], BF16, name="W1_t")
        W2_t = res.tile([P, NMT, D], BF16, name="W2_t")
        xtf = res.tile([P, NSQT, D], F32, name="xtf")

        with ExitStack() as PH:
            p1 = PH.enter_context(
                tc.tile_pool(name="p1", bufs=8, space="PSUM"))
            p1s = PH.enter_context(tc.tile_pool(name="p1s", bufs=3))
            xtb, xtb_free = tc.tile([P, NST, D], F8, name="xtb")
            for c in range(4):
                nc.scalar.dma_start(xtb[:, 4 * c:4 * c + 4, :],
                                    xtok8[:, 4 * c:4 * c + 4, :])

            # ---------- LN1 stats (token-major fp8) ----------
            st6 = res.tile([P, NST, 6], F32, name="st6")
            for st in range(NST):
                nc.vector.bn_stats(st6[:, st, :], xtb[:, st, :])
                nc.vector.bn_aggr(mv[:, :, st:st + 1], st6[:, st, :])
            xtb_free()

            def rstd_newton(nx, dst, var_ap, gain, tmp_pool, n):
                # dst = gain * var'^(-1/2), var' = var*D/(D-1) ~ 1.0.
                # Quadratic seed + one Newton step (err ~1e-4 on [0.6,1.6]).
                vp = tmp_pool.tile([nx, n], F32, name="vp", tag="nw0",
                                   bufs=1)
                y0 = tmp_pool.tile([nx, n], F32, name="y0", tag="nw1",
                                   bufs=1)
                t = tmp_pool.tile([nx, n], F32, name="t", tag="nw2", bufs=1)
                nc.vector.tensor_scalar_mul(vp, var_ap, float(D) / (D - 1))
                nc.vector.tensor_scalar(y0, vp, 0.375, -1.25,
                                        op0=MUL, op1=ADD)
                nc.vector.tensor_mul(y0, y0, vp)
                nc.vector.tensor_scalar_add(y0, y0, 1.875)
                for _ in range(2):
                    nc.vector.tensor_mul(t, y0, y0)
                    nc.vector.tensor_mul(t, t, vp)
                    nc.vector.tensor_scalar(t, t, -0.5, 1.5,
                                            op0=MUL, op1=ADD)
                    nc.vector.tensor_mul(y0, y0, t)
                nc.vector.tensor_scalar_mul(dst, y0, gain)

            # transpose mean/var to row space [16,128], then all LN1
            # math there (keeps the PE queue free for QKV right after).
            mrows = p1s.tile([NST, P], F32, name="mrows", tag="mr", bufs=1)
            vrows = p1s.tile([NST, P], F32, name="vrows", tag="vr", bufs=1)
            for srcc, dstt in ((mv[:, 0, :], mrows), (mv[:, 1, :], vrows)):
                tp = p1.tile([NST, P], F32, name="tp", tag="p1")
                nc.tensor.transpose(tp, srcc, ident)
                nc.vector.tensor_copy(dstt, tp)
            # sc_r = g1/(std*SCL) rows; bi_r = -mean*g1/std + be1 rows
            rstd_newton(NST, sc_r, vrows, g1 / SCL, p1s, P)
            nc.vector.tensor_mul(bi_r, mrows, sc_r)
            nc.vector.tensor_scalar(bi_r, bi_r, -SCL, float(be1),
                                    op0=MUL, op1=ADD)
            # sc/bi cols for the V fixup (transpose back)
            tpb = p1.tile([P, NST], F32, name="tpb", tag="p1")
            nc.tensor.transpose(tpb, sc_r, ident[0:NST, 0:NST])
            nc.vector.tensor_copy(sc_all, tpb)
            bi_all = res.tile([P, NST], F32, name="bi_all")
            tpc = p1.tile([P, NST], F32, name="tpc", tag="p1")
            nc.tensor.transpose(tpc, bi_r, ident[0:NST, 0:NST])
            nc.vector.tensor_copy(bi_all, tpc)

            def bi_col(st):
                return bi_all[:, st:st + 1]
            for c in range(4):
                for rw, scr, dst in ((sc_r, scr_sc, scale_b),
                                     (bi_r, scr_bi, bias_b)):
                    nc.sync.dma_start(scr.ap()[4 * c:4 * c + 4, :],
                                      rw[4 * c:4 * c + 4, :])
                    nc.sync.dma_start(
                        dst[:, c * 512:(c + 1) * 512],
                        bass.AP(tensor=scr.ap().tensor,
                                offset=scr.ap().offset + c * 512,
                                ap=[[0, P], [1, 512]]))

            # deferred bulk DMAs (behind the critical loads on each queue)
            nc.gpsimd.dma_start(Wo_t, Wo8)
            nc.gpsimd.dma_start(W1_t, W18)
            nc.gpsimd.dma_start(W2_t, W28)
            nc.sync.dma_start(xtf, xtokf)

            # ---------- QKV (pure fp8 DoubleRow; fixup on DVE/Scalar) ---
            def qk_feat(Wt, wsum, dstT, j, c):
                # feature-major: out[dim 128, tok 512]
                ps = p1.tile([P, 512], F32, name="ps_qk", tag="p1")
                for i in range(2):
                    nc.tensor.matmul(
                        ps, Wt[:, 2 * i:2 * i + 2, j * P:(j + 1) * P],
                        x8[:, 2 * i:2 * i + 2, c * 512:(c + 1) * 512],
                        start=(i == 0), stop=(i == 1), perf_mode=DR)
                t = p1s.tile([P, 512], F32, name="fx", tag="fx")
                nc.vector.tensor_mul(t, ps,
                                     scale_b[:, c * 512:(c + 1) * 512])
                nc.vector.scalar_tensor_tensor(
                    dstT[j][:, c * 512:(c + 1) * 512],
                    bias_b[:, c * 512:(c + 1) * 512],
                    wsum[:, j:j + 1], t, op0=MUL, op1=ADD)

            def v_tok(st):
                # token-major: out[tok 128, vdim 512]
                ps = p1.tile([P, D], F32, name="ps_v", tag="p1")
                for i in range(2):
                    nc.tensor.matmul(
                        ps, x8[:, 2 * i:2 * i + 2, st * P:(st + 1) * P],
                        Wv_t[:, 2 * i:2 * i + 2, :],
                        start=(i == 0), stop=(i == 1), perf_mode=DR)
                t = p1s.tile([P, D], F32, name="fxv", tag="fx")
                nc.vector.tensor_scalar_mul(t, ps, sc_all[:, st:st + 1])
                dst = bass.AP(
                    tensor=vo8.tensor, offset=vo8.offset + st * H * 65,
                    ap=[vo8.ap[0], [65, H], [1, DK]])
                nc.vector.scalar_tensor_tensor(
                    dst, wvs_b.rearrange("p (h c) -> p h c", c=DK),
                    bi_col(st), t.rearrange("p (h c) -> p h c", c=DK),
                    op0=MUL, op1=ADD)

            for c in range(4):
                qk_feat(Wk_t, wks_sb, kT, 0, c)
            for c in range(2):
                qk_feat(Wq_t, wqs_sb, qT, 0, c)
            for st in range(6):
                v_tok(st)
            for j in range(1, NJ):
                for c in range(4):
                    qk_feat(Wk_t, wks_sb, kT, j, c)
                for c in range(2):
                    qk_feat(Wq_t, wqs_sb, qT, j, c)
                for st in range(6 + (j - 1) * 4, min(NST, 6 + j * 4)):
                    v_tok(st)
            for st in range(14, NST):
                v_tok(st)

        # ================= attention =================
        GRP = [(0, 3), (3, 3), (6, 3), (9, 3), (12, 3), (15, 1)]
        with ExitStack() as PA:
            ps_sg = PA.enter_context(
                tc.tile_pool(name="ps_sg", bufs=2, space="PSUM"))
            ps_acc = PA.enter_context(
                tc.tile_pool(name="ps_acc", bufs=2, space="PSUM"))
            sb_pg = PA.enter_context(tc.tile_pool(name="sb_pg", bufs=2))
            sb_nrm = PA.enter_context(tc.tile_pool(name="sb_nrm", bufs=1))

            for qc in range(2):
                for hp in range(4):
                    accs = [ps_acc.tile([65, 512], F32, name=f"acc{h2}",
                                        tag="acc") for h2 in range(2)]
                    for g0, gn in GRP:
                        for h2 in range(2):
                            h = 2 * hp + h2
                            bp = 64 * h2
                            sg = ps_sg.tile([P, gn, 512], F32, name="sg",
                                            tag="sg")
                            for i in range(gn):
                                kt = g0 + i
                                nc.tensor.matmul(
                                    sg[:, i, :],
                                    kT[hp][bp:bp + DK,
                                           kt * P:(kt + 1) * P],
                                    qT[hp][bp:bp + DK,
                                           qc * 512:(qc + 1) * 512])
                            pg = sb_pg.tile([P, gn, 512], BF16, name="pg",
                                            tag="pg", bufs=6)
                            nc.scalar.activation(
                                pg.rearrange("p a b -> p (a b)"),
                                sg.rearrange("p a b -> p (a b)"), AF.Exp)
                            for i in range(gn):
                                kt = g0 + i
                                nc.tensor.matmul(
                                    accs[h2],
                                    vo8[:, kt, h * 65:h * 65 + 65],
                                    pg[:, i, :],
                                    start=(kt == 0), stop=(kt == NST - 1))
                    # normalize: all base-partition-0 (HW requirement for
                    # custom-DVE/partition_broadcast); only DVE out shifted.
                    for h2 in range(2):
                        den = sb_nrm.tile([1, 512], F32, name="den",
                                          tag=f"den{h2}")
                        rec = sb_nrm.tile([1, 512], F32, name="rec",
                                          tag=f"rec{h2}")
                        rbh = sb_nrm.tile([64, 512], F32, name="rbh",
                                          tag=f"rb{h2}")
                        nc.vector.tensor_copy(den[0:1, :],
                                              accs[h2][DK:DK + 1, :])
                        nc.vector.reciprocal_approx_fast(rec[0:1, :],
                                                         den[0:1, :])
                        nc.gpsimd.partition_broadcast(
                            rbh[0:64, :], rec[0:1, :], channels=64)
                        nc.vector.tensor_mul(
                            oT8[qc][64 * h2:64 * h2 + 64, hp, :],
                            accs[h2][0:DK, :], rbh[0:64, :])

        for f in reversed(_frees):
            f()

        # ================= Wo + LN2 + FFN =================
        with ExitStack() as PF:
            pf = PF.enter_context(
                tc.tile_pool(name="pf", bufs=4, space="PSUM"))
            pf1 = PF.enter_context(
                tc.tile_pool(name="pf1", bufs=2, space="PSUM"))
            fs = PF.enter_context(tc.tile_pool(name="fs", bufs=2))
            ff8 = fs.tile([P, NMT, SQ], BF16, name="ff8", tag="ff", bufs=1)

            # Wo token-major + residual + LN2 stats (both qc)
            for sq in range(NSQT):
                qc, sl = sq // 4, sq % 4
                ps = pf.tile([P, D], F32, name="ps_wo", tag="f")
                for j in range(NJ):
                    nc.tensor.matmul(
                        ps, oT8[qc][:, j, sl * P:(sl + 1) * P],
                        Wo_t[:, j, :], start=(j == 0), stop=(j == NJ - 1))
                nc.vector.tensor_add(x2tok[sq], ps, xtf[:, sq, :])
                st6b = fs.tile([P, 6], F32, name="st6b", tag="st6b")
                nc.vector.bn_stats(st6b, x2tok[sq])
                nc.vector.bn_aggr(mv2[:, :, sq:sq + 1], st6b)
            # LN2 scale/bias (per-token = per-partition), all 8 tiles
            rstd_newton(P, sc2, mv2[:, 1, :], g2, fs, NSQT)
            nc.vector.tensor_mul(bi2, mv2[:, 0, :], sc2)
            nc.vector.tensor_scalar(bi2, bi2, -1.0, float(be2),
                                    op0=MUL, op1=ADD)
            # LN2 apply + transpose, pipelined with FFN1 per 512-chunk
            for c in range(2):
                for sl in range(4):
                    sq = c * 4 + sl
                    xn2 = fs.tile([P, D], BF16, name="xn2", tag="xn2")
                    nc.vector.tensor_scalar(xn2, x2tok[sq],
                                            sc2[:, sq:sq + 1],
                                            bi2[:, sq:sq + 1],
                                            op0=MUL, op1=ADD)
                    for j in range(NJ):
                        pt = pf.tile([P, P], BF16, name="ptt", tag="f")
                        nc.tensor.transpose(pt, xn2[:, j * P:(j + 1) * P],
                                            identb)
                        nc.vector.tensor_copy(
                            xn2T8[:, j, sq * P:(sq + 1) * P], pt)
                for mt in range(NMT):
                    ps = pf1.tile([P, 512], F32, name="ps_f1", tag="f1")
                    for j in range(NJ):
                        nc.tensor.matmul(
                            ps, W1_t[:, j, mt * P:(mt + 1) * P],
                            xn2T8[:, j, c * 512:(c + 1) * 512],
                            start=(j == 0), stop=(j == NJ - 1))
                    nc.scalar.activation(
                        ff8[:, mt, c * 512:(c + 1) * 512], ps, AF.Relu)
            # FFN2 (bf16) + residual + store
            for sq in range(NSQT):
                ps = pf.tile([P, D], F32, name="ps_f2", tag="f")
                for mt in range(NMT):
                    nc.tensor.matmul(
                        ps, ff8[:, mt, sq * P:(sq + 1) * P],
                        W2_t[:, mt, :],
                        start=(mt == 0), stop=(mt == NMT - 1))
                ot = fs.tile([P, D], F32, name="ot", tag="ot")
                nc.vector.tensor_add(ot, ps, x2tok[sq])
                nc.sync.dma_start(out[sq * P:(sq + 1) * P, :], ot)

    nc.compile()
    return nc


def _fast_path_ok(inputs):
    if not np.all(np.asarray(inputs["src_mask"]) != 0):
        return False
    for b in ("bq", "bk", "bv", "bo", "b1", "b2"):
        if np.any(np.asarray(inputs[b]) != 0):
            return False
    if float(np.asarray(inputs["g1"])) <= 0 or float(np.asarray(inputs["g2"])) <= 0:
        return False
    return True


def _fp8(a):
    import ml_dtypes
    return np.ascontiguousarray(
        np.clip(a, -240.0, 240.0).astype(ml_dtypes.float8_e4m3))


def _w_layout(w, nsub):
    # [K, M] -> [128, nsub, M] with k = j*128 + p
    k, m = w.shape
    return np.ascontiguousarray(
        w.reshape(nsub, P, m).transpose(1, 0, 2))


def kernel(**inputs):
    x = np.ascontiguousarray(np.asarray(inputs["x"], np.float32))
    g1 = float(np.asarray(inputs["g1"]))
    be1 = float(np.asarray(inputs["be1"]))
    g2 = float(np.asarray(inputs["g2"]))
    be2 = float(np.asarray(inputs["be2"]))

    if not _fast_path_ok(inputs):
        return _np_reference(**{k: np.asarray(v) for k, v in inputs.items()})

    from concourse.bass_utils import run_bass_kernel_spmd
    import ml_dtypes

    key = (g1, be1, g2, be2)
    if key not in _CACHE:
        _CACHE[key] = _build(*key)
    nc = _CACHE[key]

    scale = np.float32(1.0 / np.sqrt(DK))
    Wq8 = _fp8(_w_layout(np.asarray(inputs["Wq"], np.float32) * (scale * SCL), NJ))
    Wk8 = _fp8(_w_layout(np.asarray(inputs["Wk"], np.float32) * SCL, NJ))
    Wv8 = _fp8(_w_layout(np.asarray(inputs["Wv"], np.float32) * SCL, NJ))
    BFD = ml_dtypes.bfloat16
    Wo8 = np.ascontiguousarray(
        _w_layout(np.asarray(inputs["Wo"], np.float32), NJ).astype(BFD))
    W18 = np.ascontiguousarray(
        _w_layout(np.asarray(inputs["W1"], np.float32), NJ).astype(BFD))
    W28 = np.ascontiguousarray(
        _w_layout(np.asarray(inputs["W2"], np.float32), NMT).astype(BFD))

    def colsum(w8):
        return np.ascontiguousarray(
            (w8.astype(np.float32).sum(axis=(0, 1)) / SCL)[None, :])

    wqs = colsum(Wq8)
    wks = colsum(Wk8)
    wvs = colsum(Wv8)

    in_maps = []
    for c in range(8):
        b, hh = c // 2, c % 2
        if hh == 0:
            xp = x[b]
        else:
            xp = np.concatenate([x[b, SQ:], x[b, :SQ]], axis=0)
        xp = np.ascontiguousarray(xp)
        x8 = _fp8(_w_layout(xp.T.copy(), NJ))
        xt8 = _fp8(xp.reshape(NST, P, D).transpose(1, 0, 2))
        xtf_l = np.ascontiguousarray(
            xp[:SQ].reshape(NSQT, P, D).transpose(1, 0, 2))
        in_maps.append(dict(
            x8=np.ascontiguousarray(x8),
            xtok8=xt8,
            xtokf=xtf_l,
            Wq8=Wq8, Wk8=Wk8, Wv8=Wv8, Wo8=Wo8, W18=W18, W28=W28,
            wqs=wqs, wks=wks, wvs=wvs))

    res = run_bass_kernel_spmd(nc, in_maps, core_ids=list(range(8)),
                               trace=_TRACE["trace"],
                               trace_cores=_TRACE["trace_cores"])
    _LAST["res"] = res

    full = np.empty((B, S, D), np.float32)
    for c in range(8):
        b, hh = c // 2, c % 2
        full[b, hh * SQ:(hh + 1) * SQ] = res.results[c]["out"]
    return full
